# revision 32
# baseline (speedup 1.0000x reference)
"""Deformable group-correlation kernel for TRN2 (8 NeuronCores).

Reference op: bilinear-sample right_feature at per-pixel coords
(base grid + flow + 1x9 window offsets + extra offsets), then group-wise
(4 groups x 64ch) mean of left * sampled -> (2, 36, 80, 160).

Sharding: 8 cores = (batch b in {0,1}) x (h-quarter hq in {0..3}).
Each core: all 256 channels, 20 pixel rows = 3200 pixels = 25 strips of 128.

Per-core pipeline (25 strips, 1152 samples each = 9 search-pos x 128 pixels):
  - host: zero-padded channel-last 4-corner patch table
    r4[(y,x), (c64,g,k)] bf16 (2KB granules) + precomputed gather indices
    (wrap, 16-partition wrapped) and bilinear weights (w4), so the device
    does no coordinate math.
  - gpsimd dma_gather (flat, transpose=False, alternating between 2 SWDGE
    queues so descriptor-gen overlaps the previous gather's drain): sample
    i = s*128+pp lands contiguously: patch[pp, s, 1024].
  - row layout is interleaved j = c*16 + g*4 + k, so the channel-sum tree
    is contiguous halving adds.
  - DVE: in-place patch *= lt (left/64, replicated over k), tree adds
    (bf16 2x-mode) down to the 16 (g,k) sums, * w4, sum over k
    -> corr[pp, (s,g)] f32.
  - scalar engine DMAs corr out.
"""

import sys

sys.path.insert(0, "/opt/trn_rl_repo")

from contextlib import ExitStack

import numpy as np
import ml_dtypes

from concourse import bacc, bass, mybir
from concourse.bass_utils import run_bass_kernel_spmd
from concourse.library_config import mlp as mlp_library

F32 = mybir.dt.float32
BF16 = mybir.dt.bfloat16
I16 = mybir.dt.int16
AF = mybir.AluOpType
AX = mybir.AxisListType

B, C, H, W = 2, 256, 80, 160
G, gC, S = 4, 64, 9
PADDING = 2
TAB_H, TAB_W = 84, 164  # table: y in [0,84), x in [0,164); row = y*164 + x
NROWS = TAB_H * TAB_W  # 13776 granule rows
ELEM = 4 * C  # 1024 bf16 per granule = 2KB (4 groups x 4 corners x 64 ch)
HQ = H // 4  # 20 rows per core
NSTRIP = HQ * W // 128  # 25 strips of 128 pixels
NI = S * 128  # 1152 samples per strip
NW = NI // 16  # 72 wrapped-index columns per strip
M = S * 16  # 144 (s, g, k) groups per strip, 64 channels each
MAGIC = 8388608.0  # 2**23

NPATCH = 6  # patch buffer count
LT_SPLIT = 6  # strips covered by the first lt chunk

_graph_cache = {}


def _build_graph():
    nc = bacc.Bacc(
        "TRN2",
        detect_race_conditions=False,
        num_swdge_queues=2,
        target_bir_lowering=True,
    )

    r4 = nc.declare_dram_parameter("r4", [NROWS, ELEM], BF16, isOutput=False)
    lt = nc.declare_dram_parameter("lt", [128, NSTRIP * ELEM], BF16, isOutput=False)
    w4 = nc.declare_dram_parameter("w4", [128, NSTRIP * 36], F32, isOutput=False)
    wrap = nc.declare_dram_parameter("wrap", [128, NSTRIP * NW], I16, isOutput=False)
    out = nc.declare_dram_parameter("out", [NSTRIP * 128, 36], F32, isOutput=True)

    with ExitStack() as stk:
        sb = lambda name, shape, dt: stk.enter_context(nc.sbuf_tensor(name, shape, dt))
        wrap_s = sb("wrap_s", [128, NSTRIP * NW], I16)
        lt_s = sb("lt_s", [128, NSTRIP * ELEM], BF16)
        w4_s = sb("w4_s", [128, NSTRIP * 36], F32)
        patch_bufs = [sb(f"patch{i}", [128, S * ELEM], BF16) for i in range(NPATCH)]
        red = sb("red", [128, M], F32)
        t3 = sb("t3", [128, M], F32)
        corrS = [sb("corrS0", [128, 36], F32), sb("corrS1", [128, 36], F32)]
        sem = lambda name: stk.enter_context(nc.semaphore(name))
        wrap_sem = sem("wrap_sem")
        gat0A_sem = sem("gat0A_sem")
        gat0B_sem = sem("gat0B_sem")
        ltA_sem = sem("ltA_sem")
        w4_sem = sem("w4_sem")
        ltB_sem = sem("ltB_sem")
        gat_sems = [sem(f"gat_sem{i}") for i in range(NPATCH)]
        dve_sem = sem("dve_sem")
        corr_sem = sem("corr_sem")
        out_sems = [sem("out_sem0"), sem("out_sem1")]
        patches = patch_bufs

        # patch view helper: t[pp, s*1024 + q]
        def tv(pb, off, dims):
            return bass.AP(pb, off, dims)

        with nc.Block() as block:

            @block.sync
            def _(sync):
                sync.dma_start(wrap_s[:, :], wrap[:, :]).then_inc(wrap_sem, 16)
                sync.dma_start(
                    lt_s[:, : LT_SPLIT * ELEM], lt[:, : LT_SPLIT * ELEM]
                ).then_inc(ltA_sem, 16)
                sync.dma_start(w4_s[:, :], w4[:, :]).then_inc(w4_sem, 16)
                sync.dma_start(
                    lt_s[:, LT_SPLIT * ELEM :], lt[:, LT_SPLIT * ELEM :]
                ).then_inc(ltB_sem, 16)

            @block.gpsimd
            def _(gpsimd):
                gpsimd.load_library(mlp_library)
                gpsimd.wait_ge(wrap_sem, 16)

                def gather(n):
                    if n >= NPATCH:
                        # patch[n % NPATCH] free after DVE l2(n - NPATCH);
                        # gpsimd's own tail(n - NPATCH) is earlier in program
                        # order so no extra wait needed for it.
                        gpsimd.wait_ge(dve_sem, n - NPATCH + 1)
                    pb = patches[n % NPATCH]
                    dst = bass.AP(pb, 0, [[S * ELEM, 128], [ELEM, S], [1, ELEM]])
                    idxs_ap = wrap_s[:, n * NW : (n + 1) * NW]
                    gpsimd.dma_gather(
                        dst,
                        r4[:, :],
                        idxs_ap,
                        NI,
                        NI,
                        ELEM,
                        transpose=False,
                        single_packet=False,
                        queue_num=n % 2,
                    ).then_inc(gat_sems[n % NPATCH], 16)

                # strip 0 in two halves so the DVE can start sooner
                pb0 = patches[0]
                dstA = bass.AP(pb0, 0, [[S * ELEM, 128], [ELEM, 4], [1, ELEM]])
                gpsimd.dma_gather(
                    dstA, r4[:, :], wrap_s[:, 0:32], 512, 512, ELEM,
                    transpose=False, single_packet=False, queue_num=0,
                ).then_inc(gat0A_sem, 16)
                dstB = bass.AP(
                    pb0, 4 * ELEM, [[S * ELEM, 128], [ELEM, 5], [1, ELEM]]
                )
                gpsimd.dma_gather(
                    dstB, r4[:, :], wrap_s[:, 32:72], 640, 640, ELEM,
                    transpose=False, single_packet=False, queue_num=1,
                ).then_inc(gat0B_sem, 16)
                for n in range(1, NSTRIP):
                    gather(n)

            @block.vector
            def _(vector):
                vector.wait_ge(ltA_sem, 16)
                vector.wait_ge(w4_sem, 16)
                pb0 = patches[0]
                for sub in range(2):
                    s0, ns = (0, 4) if sub == 0 else (4, 5)
                    base = s0 * ELEM
                    if sub == 0:
                        vector.wait_ge(gat0A_sem, 16)
                    else:
                        vector.wait_ge(gat0B_sem, 16)
                    o = tv(pb0, base, [[S * ELEM, 128], [ELEM, ns], [1, ELEM]])
                    i1 = bass.AP(
                        lt_s, 0, [[NSTRIP * ELEM, 128], [0, ns], [1, ELEM]]
                    )
                    vector.tensor_tensor(out=o, in0=o, in1=i1, op=AF.mult)
                    for half in (512, 256, 128, 64, 32):
                        o = tv(pb0, base, [[S * ELEM, 128], [ELEM, ns], [1, half]])
                        i1h = tv(
                            pb0, base + half, [[S * ELEM, 128], [ELEM, ns], [1, half]]
                        )
                        vector.tensor_tensor(out=o, in0=o, in1=i1h, op=AF.add)
                    o = bass.AP(red, s0 * 16, [[M, 128], [16, ns], [1, 16]])
                    i0 = tv(pb0, base, [[S * ELEM, 128], [ELEM, ns], [1, 16]])
                    i1h = tv(pb0, base + 16, [[S * ELEM, 128], [ELEM, ns], [1, 16]])
                    mm = vector.tensor_tensor(out=o, in0=i0, in1=i1h, op=AF.add)
                    if sub == 1:
                        mm.then_inc(dve_sem, 1)
                    o = bass.AP(t3, s0 * 16, [[M, 128], [16, ns], [4, 4], [1, 4]])
                    i0 = bass.AP(red, s0 * 16, [[M, 128], [16, ns], [4, 4], [1, 4]])
                    i1h = bass.AP(
                        w4_s, s0 * 4, [[NSTRIP * 36, 128], [4, ns], [0, 4], [1, 4]]
                    )
                    vector.tensor_tensor(out=o, in0=i0, in1=i1h, op=AF.mult)
                    co = bass.AP(corrS[0], s0 * 4, [[36, 128], [1, ns * 4]])
                    ti = bass.AP(t3, s0 * 16, [[M, 128], [4, ns * 4], [1, 4]])
                    mm = vector.tensor_reduce(co, ti, axis=AX.X, op=AF.add)
                    if sub == 1:
                        mm.then_inc(corr_sem, 1)
                for n in range(1, NSTRIP):
                    if n == LT_SPLIT:
                        vector.wait_ge(ltB_sem, 16)
                    vector.wait_ge(
                        gat_sems[n % NPATCH],
                        16 * (n // NPATCH) if n % NPATCH == 0 else 16 * (n // NPATCH + 1),
                    )
                    pb = patches[n % NPATCH]
                    # patch *= lt (in place)
                    o = tv(pb, 0, [[S * ELEM, 128], [ELEM, S], [1, ELEM]])
                    i1 = bass.AP(
                        lt_s, n * ELEM, [[NSTRIP * ELEM, 128], [0, S], [1, ELEM]]
                    )
                    vector.tensor_tensor(out=o, in0=o, in1=i1, op=AF.mult)
                    # tree: contiguous halves 512 ... 32
                    for half in (512, 256, 128, 64, 32):
                        o = tv(pb, 0, [[S * ELEM, 128], [ELEM, S], [1, half]])
                        i0 = tv(pb, 0, [[S * ELEM, 128], [ELEM, S], [1, half]])
                        i1 = tv(pb, half, [[S * ELEM, 128], [ELEM, S], [1, half]])
                        vector.tensor_tensor(out=o, in0=i0, in1=i1, op=AF.add)
                    # final pair -> f32 red[pp, (s, gk)]; patch buffer now free
                    o = bass.AP(red, 0, [[M, 128], [16, S], [1, 16]])
                    i0 = tv(pb, 0, [[S * ELEM, 128], [ELEM, S], [1, 16]])
                    i1 = tv(pb, 16, [[S * ELEM, 128], [ELEM, S], [1, 16]])
                    vector.tensor_tensor(out=o, in0=i0, in1=i1, op=AF.add).then_inc(
                        dve_sem, 1
                    )
                    # t3 = red * w4 (w4 broadcast over g)
                    o = bass.AP(t3, 0, [[M, 128], [16, S], [4, 4], [1, 4]])
                    i0 = bass.AP(red, 0, [[M, 128], [16, S], [4, 4], [1, 4]])
                    i1 = bass.AP(
                        w4_s, n * 36, [[NSTRIP * 36, 128], [4, S], [0, 4], [1, 4]]
                    )
                    vector.tensor_tensor(out=o, in0=i0, in1=i1, op=AF.mult)
                    # corr[pp, (s,g)] = sum_k t3
                    if n >= 2:
                        vector.wait_ge(out_sems[n % 2], 16 * ((n - 2) // 2 + 1))
                    co = bass.AP(corrS[n % 2], 0, [[36, 128], [1, 36]])
                    ti = bass.AP(t3, 0, [[M, 128], [16, S], [4, 4], [1, 4]])
                    vector.tensor_reduce(co, ti, axis=AX.X, op=AF.add).then_inc(
                        corr_sem, 1
                    )

            @block.scalar
            def _(scalar):
                for n in range(NSTRIP):
                    scalar.wait_ge(corr_sem, n + 1)
                    dst = out[n * 128 : (n + 1) * 128, :]
                    scalar.dma_start(dst, corrS[n % 2][:, :]).then_inc(
                        out_sems[n % 2], 16
                    )
                scalar.wait_ge(out_sems[0], 16 * ((NSTRIP + 1) // 2))
                scalar.wait_ge(out_sems[1], 16 * (NSTRIP // 2))

    if not nc.is_finalized():
        nc.finalize()
    return nc


def _host_prep(left_feature, right_feature, flow, extra_offset):
    """Per-core inputs. Core ordering: core = b*4 + hq."""
    lf = np.asarray(left_feature, np.float32)
    rf = np.asarray(right_feature, np.float32)
    fl = np.asarray(flow, np.float32)
    eo = np.asarray(extra_offset, np.float32)

    p_idx = np.arange(128)
    strip = np.arange(NSTRIP)
    pi = strip[:, None] * 128 + p_idx[None, :]  # [25, 128] pixel within quarter
    hl = pi // W
    w = pi % W

    offx = np.arange(S, dtype=np.float32) - 4.0

    in_maps = []
    for b in range(B):
        rp = np.zeros((TAB_H + 1, TAB_W + 1, C), np.float32)
        rp[PADDING : PADDING + H, PADDING : PADDING + W] = rf[b].transpose(1, 2, 0)
        # corners k: (dy,dx) = (k//2, k%2); row layout (g, k, c64)
        corn = np.stack(
            [
                rp[0:TAB_H, 0:TAB_W],
                rp[0:TAB_H, 1 : TAB_W + 1],
                rp[1 : TAB_H + 1, 0:TAB_W],
                rp[1 : TAB_H + 1, 1 : TAB_W + 1],
            ],
            axis=2,
        )  # [84, 164, 4k, 256c]
        r4_np = np.ascontiguousarray(
            corn.reshape(TAB_H, TAB_W, 4, G, gC)
            .transpose(0, 1, 4, 3, 2)  # (y, x, c, g, k)
            .reshape(NROWS, ELEM)
            .astype(ml_dtypes.bfloat16)
        )

        for hq in range(4):
            h = hq * HQ + hl  # [25, 128] global h
            fx = fl[b, 0][h, w]
            fy = fl[b, 1][h, w]
            cbx = w.astype(np.float32) + fx + PADDING  # [25, 128]
            cby = h.astype(np.float32) + fy + PADDING

            eo_b = eo[b].reshape(S, 2, H, W)
            exx = eo_b[:, 0][:, h, w] + offx[:, None, None]  # [S, 25, 128]
            exy = eo_b[:, 1][:, h, w]

            hflat = h.reshape(-1)
            wflat = w.reshape(-1)
            # lt[pp, strip, (g, k, c)] = left[b, g*64+c, pix]/64 (same for all k)
            lv = (lf[b] / gC)[:, hflat, wflat]  # [256, 3200]
            lt4 = (
                lv.reshape(G, gC, NSTRIP, 128).transpose(3, 2, 1, 0)
            )  # [128, 25, c, g]
            lt_np = np.ascontiguousarray(
                np.broadcast_to(
                    lt4[:, :, :, :, None], (128, NSTRIP, gC, G, 4)
                )
                .reshape(128, NSTRIP * ELEM)
                .astype(ml_dtypes.bfloat16)
            )

            # coords in f32 (round-to-nearest via the 2^23 trick)
            # [128p, strip, s]
            xq = np.clip(
                exx.transpose(2, 1, 0) + cbx.T[:, :, None], 0.5, TAB_W - 1.5
            ).astype(np.float32)
            yq = np.clip(
                exy.transpose(2, 1, 0) + cby.T[:, :, None], 0.5, TAB_H - 1.5
            ).astype(np.float32)
            x0 = ((xq + np.float32(MAGIC - 0.5)) + np.float32(-MAGIC)).astype(
                np.float32
            )
            y0 = ((yq + np.float32(MAGIC - 0.5)) + np.float32(-MAGIC)).astype(
                np.float32
            )
            fxw, fyw = xq - x0, yq - y0  # [128p, strip, s]
            gxw, gyw = 1.0 - fxw, 1.0 - fyw
            w4v = np.stack(
                [gxw * gyw, fxw * gyw, gxw * fyw, fxw * fyw], 0
            )  # [4k, 128, strip, s]
            # w4[pp, strip, s, k]
            w4_np = np.ascontiguousarray(
                w4v.transpose(1, 2, 3, 0).reshape(128, NSTRIP * 36).astype(np.float32)
            )

            # gather row index = y0*TAB_W + x0, wrapped: idx for i=s*128+pp at
            # [pp%16, strip*NW + s*8 + pp//16], replicated over 8 Q7 cores.
            idx = (y0 * np.float32(TAB_W) + x0).astype(np.int32)  # [128,strip,s]
            idx_r = idx.reshape(8, 16, NSTRIP, S)  # [a=pp//16, m=pp%16, n, s]
            wrap_np = np.ascontiguousarray(
                np.tile(
                    idx_r.transpose(1, 2, 3, 0).reshape(16, NSTRIP * NW), (8, 1)
                ).astype(np.int16)
            )

            in_maps.append(
                {
                    "r4": r4_np,
                    "lt": lt_np,
                    "w4": w4_np,
                    "wrap": wrap_np,
                }
            )
    return in_maps


def kernel(**inputs):
    if "nc" not in _graph_cache:
        _graph_cache["nc"] = _build_graph()
    nc = _graph_cache["nc"]

    in_maps = _host_prep(
        inputs["left_feature"],
        inputs["right_feature"],
        inputs["flow"],
        inputs["extra_offset"],
    )
    res = run_bass_kernel_spmd(nc, in_maps, core_ids=list(range(8)))
    _graph_cache["last_res"] = res
    outs = [r["out"] for r in res.results]

    full = np.zeros((B, G * S, H, W), np.float32)
    for core in range(8):
        b, hq = divmod(core, 4)
        # out rows: [strip, pp], cols: [s, g]
        o = np.asarray(outs[core], np.float32).reshape(NSTRIP, 128, S, G)
        o = o.transpose(3, 2, 0, 1).reshape(G, S, HQ, W)
        for g in range(G):
            for s in range(S):
                full[b, g * S + s, hq * HQ : (hq + 1) * HQ, :] = o[g, s]
    return full
